# revision 5
# baseline (speedup 1.0000x reference)
"""Trainium2 Bass kernel for nn_CrossLayer (B=8, C=256, S=2048, D=64).

Reference computation (per batch b):
    scores = b_i @ c_i^T               [S, S]
    mid    = softmax(scores, axis=-1)  row softmax over m
    out    = a_i @ mid^T + a_i         [C, S]

Strategy: data-parallel over the batch dim — one batch per NeuronCore
(8 cores).  Per core, everything is computed in the "column" layout
scores^T[m, n] so that the softmax contraction axis m sits on SBUF
partitions, which is what the second matmul needs:

    phase 1: scoresT tiles [128m, 2048n] = cT-tile.T @ bT  (K=D=64, f32)
             E = exp(scoresT)  (no max subtraction needed: |scores| <~ 50,
             exp stays well inside f32/bf16 range), stored bf16 in SBUF.
    Z[n]   = ones.T @ E        (PE partition-sum of E over m)
    out2   = aT.T @ E          (K=m accumulated over 16 m-tiles, bf16)
    out    = out2 * (1/Z) + a  (DVE), with 1/Z broadcast across
             partitions via a K=1 ones matmul.

Host side transposes the per-batch slices (b^T, c^T, a^T) so the device
never has to transpose anything.
"""

import os
from contextlib import ExitStack

import numpy as np
import ml_dtypes

import concourse.bass as bass
import concourse.tile as tile
from concourse import mybir
from concourse.vector_clock import ScopedClock, VectorClock
from concourse.bass_utils import run_bass_kernel_spmd

F32 = mybir.dt.float32
BF16 = mybir.dt.bfloat16

B, C, S, D = 8, 256, 2048, 64
N_CORES = 8
MT = S // 128  # m-tiles of scores^T (partition tiles)
NCH = S // 512  # n chunks for phase 2 / PSUM banks

# softmax intermediate dtype (bf16 halves PE time + SBUF; f32 fallback for
# accuracy if ever needed)
E_DT = BF16 if os.environ.get("KERNEL_E_DT", "bf16") == "bf16" else F32
E_NP = ml_dtypes.bfloat16 if E_DT == BF16 else np.float32


class PatchedTileContext(tile.TileContext):
    """This walrus build caps sync waits per SP Drain/NoOp at <3; the stock
    TileContext tail drain carries one wait per outstanding semaphore.
    Split them one-per-NOP before a clean drain."""

    def _drain_and_barrier(self, tick_clock, wait_clock):
        gclock = tick_clock.global_clock
        nprocs = len(gclock)
        for proc in range(nprocs):
            tick = gclock[proc]
            if tick <= 0:
                continue
            vec = [0] * nprocs
            vec[proc] = tick
            nop_inst = self.nc.sync.nop(nofuse=True)
            wait_clock.add_sem_waits(
                nop_inst.ins, ScopedClock({None: VectorClock(vec)})
            )
        self.nc.sync.drain()
        self.nc.all_engine_barrier()
        assert self.sems is not None
        popped = self.nc._tile_sem_poison_stack.pop()
        assert popped is self._sem_poison
        self.nc.clear_and_free_semaphores(list(self.sems.allocated().values()))
        self.nc.all_engine_barrier()


def _split_sync_waits_json(raw: bytes, cap: int = 1) -> bytes:
    """This walrus build rejects instructions carrying more than ~1 sync
    wait (setupSyncWait: "Too many sync wait commands").  Rewrite the BIR
    JSON so any instruction keeps at most `cap` waits and the excess move
    to NoOps injected immediately before it in the same engine stream —
    identical semantics, compiler-acceptable encoding."""
    import json

    m = json.loads(raw)
    ctr = 0
    for fn in m["functions"]:
        for bb in fn["blocks"]:
            new_insts = []
            for inst in bb["instructions"]:
                si = inst.get("sync_info") or {}
                ow = si.get("on_wait") or []
                if len(ow) > cap:
                    n_extra = len(ow) - cap
                    for w in ow[:n_extra]:
                        ctr += 1
                        nop = {
                            "engine": inst["engine"],
                            "ins": [],
                            "name": f"I-{90000 + ctr}",
                            "opcode": "NoOp",
                            "outs": [],
                            "sync_info": {"on_update": [], "on_wait": [w]},
                        }
                        if inst.get("debug") is not None:
                            nop["debug"] = inst["debug"]
                        new_insts.append(nop)
                    si["on_wait"] = ow[n_extra:]
                new_insts.append(inst)
            bb["instructions"] = new_insts
    return json.dumps(m).encode()


def build_nc() -> bass.Bass:
    nc = bass.Bass()
    bT = nc.declare_dram_parameter("bT", [D, S], F32, isOutput=False)
    cT = nc.declare_dram_parameter("cT", [D, S], F32, isOutput=False)
    aT = nc.declare_dram_parameter("aT", [S, C], E_DT, isOutput=False)
    a32 = nc.declare_dram_parameter("a32", [C, S], F32, isOutput=False)
    out = nc.declare_dram_parameter("out", [C, S], F32, isOutput=True)

    Exp = mybir.ActivationFunctionType.Exp

    with PatchedTileContext(nc) as tc, ExitStack() as ctx:
        const = ctx.enter_context(tc.tile_pool(name="const", bufs=1))
        inp = ctx.enter_context(tc.tile_pool(name="inp", bufs=1))
        epool = ctx.enter_context(tc.tile_pool(name="epool", bufs=1))

        # ---- input loads ----
        bT_sb = inp.tile([D, S], F32, tag="bT")
        nc.sync.dma_start(out=bT_sb, in_=bT[:, :])
        cT_sb = inp.tile([D, S], F32, tag="cT")
        nc.sync.dma_start(out=cT_sb, in_=cT[:, :])
        aT_sb = inp.tile([128, MT, C], E_DT, tag="aT")
        nc.sync.dma_start(out=aT_sb, in_=aT.rearrange("(t p) c -> p t c", p=128))
        a32_sb = inp.tile([128, C // 128, S], F32, tag="a32")
        nc.sync.dma_start(out=a32_sb, in_=a32.rearrange("(t p) n -> p t n", p=128))

        ones_col = const.tile([128, 1], E_DT, tag="ones_col")  # K=128, M=1
        nc.vector.memset(ones_col, 1.0)
        ones_row = const.tile([1, 128], F32, tag="ones_row")  # K=1, M=128
        nc.vector.memset(ones_row, 1.0)

        # persistent E^T tiles (16 x [128, S])
        e_tiles = []
        for mt in range(MT):
            e_tiles.append(epool.tile([128, S], E_DT, name=f"e{mt}", tag=f"e{mt}"))

        # ---- phase 1: scores^T -> exp ----
        with tc.tile_pool(name="scps", bufs=2, space="PSUM") as scps:
            for mt in range(MT):
                sc = scps.tile([128, S], F32, tag="sc")  # 4 banks
                for j in range(NCH):
                    nc.tensor.matmul(
                        sc[:, j * 512 : (j + 1) * 512],
                        lhsT=cT_sb[:, mt * 128 : (mt + 1) * 128],
                        rhs=bT_sb[:, j * 512 : (j + 1) * 512],
                        start=True,
                        stop=True,
                    )
                nc.scalar.activation(e_tiles[mt][:, :], sc[:, :], Exp)

        # ---- phase 2: out2 = aT.T @ E, Z = ones.T @ E, finalize ----
        with (
            tc.tile_pool(name="o2ps", bufs=2, space="PSUM") as o2ps,
            tc.tile_pool(name="zps", bufs=2, space="PSUM") as zps,
            tc.tile_pool(name="rbps", bufs=2, space="PSUM") as rbps,
            tc.tile_pool(name="fin", bufs=3) as fin,
            tc.tile_pool(name="outp", bufs=4) as outp,
        ):
            out_r = out.rearrange("(t p) n -> p t n", p=128)
            for j in range(NCH):
                js = slice(j * 512, (j + 1) * 512)
                o2_0 = o2ps.tile([128, 512], F32, tag="o2a")
                o2_1 = o2ps.tile([128, 512], F32, tag="o2b")
                z = zps.tile([1, 512], F32, tag="z")
                for mt in range(MT):
                    st = mt == 0
                    sp = mt == MT - 1
                    ejs = e_tiles[mt][:, js]
                    nc.tensor.matmul(
                        o2_0, lhsT=aT_sb[:, mt, 0:128], rhs=ejs, start=st, stop=sp
                    )
                    nc.tensor.matmul(
                        o2_1, lhsT=aT_sb[:, mt, 128:256], rhs=ejs, start=st, stop=sp
                    )
                    nc.tensor.matmul(
                        z, lhsT=ones_col[:, 0:1], rhs=ejs, start=st, stop=sp
                    )
                r = fin.tile([1, 512], F32, tag="r")
                nc.vector.reciprocal(r, z[0:1, :])
                rb = rbps.tile([128, 512], F32, tag="rb")
                nc.tensor.matmul(
                    rb, lhsT=ones_row[0:1, :], rhs=r[0:1, :], start=True, stop=True
                )
                rb_sb = fin.tile([128, 512], F32, tag="rb_sb")
                nc.scalar.copy(rb_sb, rb[:, :])
                for ct in range(C // 128):
                    o2 = o2_0 if ct == 0 else o2_1
                    t1 = fin.tile([128, 512], F32, tag="t1")
                    nc.vector.tensor_mul(t1, o2[:, :], rb_sb)
                    o_sb = outp.tile([128, 512], F32, tag="o_sb")
                    nc.vector.tensor_add(o_sb, t1, a32_sb[:, ct, js])
                    nc.sync.dma_start(out=out_r[:, ct, js], in_=o_sb)

    orig_to_json_bytes = nc.to_json_bytes

    def to_json_bytes():
        return _split_sync_waits_json(orig_to_json_bytes())

    nc.to_json_bytes = to_json_bytes
    return nc


_NC_CACHE = None


def _get_nc():
    global _NC_CACHE
    if _NC_CACHE is None:
        _NC_CACHE = build_nc()
    return _NC_CACHE


def kernel(a, b, c, **run_kwargs):
    """a: [8, 256, 2048] f32, b: [8, 2048, 64] f32, c: [8, 2048, 64] f32
    -> [8, 256, 2048] f32"""
    a = np.asarray(a, dtype=np.float32)
    b = np.asarray(b, dtype=np.float32)
    c = np.asarray(c, dtype=np.float32)
    in_maps = []
    for i in range(N_CORES):
        in_maps.append(
            {
                "bT": np.ascontiguousarray(b[i].T),
                "cT": np.ascontiguousarray(c[i].T),
                "aT": np.ascontiguousarray(a[i].T).astype(E_NP),
                "a32": np.ascontiguousarray(a[i]),
            }
        )
    res = run_bass_kernel_spmd(_get_nc(), in_maps, list(range(N_CORES)), **run_kwargs)
    out = np.stack([np.asarray(res.results[i]["out"]) for i in range(N_CORES)])
    if run_kwargs:
        kernel.last_result = res
    return out.astype(np.float32)


# revision 9
# speedup vs baseline: 1.6897x; 1.6897x over previous
"""Trainium2 Bass kernel for nn_CrossLayer (B=8, C=256, S=2048, D=64).

Reference computation (per batch b):
    scores = b_i @ c_i^T               [S, S]
    mid    = softmax(scores, axis=-1)  row softmax over m
    out    = a_i @ mid^T + a_i         [C, S]

Strategy: data-parallel over the batch dim — one batch per NeuronCore
(8 cores).  Per core, everything is computed in the "column" layout
scores^T[m, n] so that the softmax contraction axis m sits on SBUF
partitions, which is what the second matmul needs:

    phase 1: scoresT tiles [128m, 2048n] = cT-tile.T @ bT  (K=D=64, f32)
             E = exp(scoresT)  (no max subtraction needed: |scores| <~ 50,
             exp stays well inside f32/bf16 range), stored bf16 in SBUF.
    Z[n]   = ones.T @ E        (PE partition-sum of E over m)
    out2   = aT.T @ E          (K=m accumulated over 16 m-tiles, bf16)
    out    = out2 * (1/Z) + a  (DVE), with 1/Z broadcast across
             partitions via a K=1 ones matmul.

Host side transposes the per-batch slices (b^T, c^T, a^T) so the device
never has to transpose anything.
"""

import os
from contextlib import ExitStack

import numpy as np
import ml_dtypes

import concourse.bass as bass
import concourse.tile as tile
from concourse import mybir
from concourse.vector_clock import ScopedClock, VectorClock
from concourse.bass_utils import run_bass_kernel_spmd

F32 = mybir.dt.float32
BF16 = mybir.dt.bfloat16

B, C, S, D = 8, 256, 2048, 64
N_CORES = 8
MT = S // 128  # m-tiles of scores^T (partition tiles)
NCH = S // 512  # n chunks for phase 2 / PSUM banks

# All matmul operands are fp32 bitcast to float32r at the matmul call:
# f32r streams 1 cycle/row (vs 4 for plain f32) when N >= 256, with full
# fp32 data — no bf16 precision loss anywhere.
F32R = mybir.dt.float32r
E_DT = F32R
E_NP = np.float32


class PatchedTileContext(tile.TileContext):
    """This walrus build caps sync waits per SP Drain/NoOp at <3; the stock
    TileContext tail drain carries one wait per outstanding semaphore.
    Split them one-per-NOP before a clean drain."""

    def _drain_and_barrier(self, tick_clock, wait_clock):
        gclock = tick_clock.global_clock
        nprocs = len(gclock)
        for proc in range(nprocs):
            tick = gclock[proc]
            if tick <= 0:
                continue
            vec = [0] * nprocs
            vec[proc] = tick
            nop_inst = self.nc.sync.nop(nofuse=True)
            wait_clock.add_sem_waits(
                nop_inst.ins, ScopedClock({None: VectorClock(vec)})
            )
        self.nc.sync.drain()
        self.nc.all_engine_barrier()
        assert self.sems is not None
        popped = self.nc._tile_sem_poison_stack.pop()
        assert popped is self._sem_poison
        self.nc.clear_and_free_semaphores(list(self.sems.allocated().values()))
        self.nc.all_engine_barrier()


def _split_sync_waits_json(raw: bytes, cap: int = 1) -> bytes:
    """This walrus build rejects instructions carrying more than ~1 sync
    wait (setupSyncWait: "Too many sync wait commands").  Rewrite the BIR
    JSON so any instruction keeps at most `cap` waits and the excess move
    to NoOps injected immediately before it in the same engine stream —
    identical semantics, compiler-acceptable encoding."""
    import json

    m = json.loads(raw)
    ctr = 0
    for fn in m["functions"]:
        for bb in fn["blocks"]:
            new_insts = []
            for inst in bb["instructions"]:
                si = inst.get("sync_info") or {}
                ow = si.get("on_wait") or []
                if len(ow) > cap:
                    n_extra = len(ow) - cap
                    for w in ow[:n_extra]:
                        ctr += 1
                        nop = {
                            "engine": inst["engine"],
                            "ins": [],
                            "name": f"I-{90000 + ctr}",
                            "opcode": "NoOp",
                            "outs": [],
                            "sync_info": {"on_update": [], "on_wait": [w]},
                        }
                        if inst.get("debug") is not None:
                            nop["debug"] = inst["debug"]
                        new_insts.append(nop)
                    si["on_wait"] = ow[n_extra:]
                new_insts.append(inst)
            bb["instructions"] = new_insts
    return json.dumps(m).encode()


def build_nc() -> bass.Bass:
    nc = bass.Bass()
    bT = nc.declare_dram_parameter("bT", [D, S], F32R, isOutput=False)
    cT = nc.declare_dram_parameter("cT", [D, S], F32R, isOutput=False)
    aT = nc.declare_dram_parameter("aT", [S, C], F32R, isOutput=False)
    a32 = nc.declare_dram_parameter("a32", [C, S], F32, isOutput=False)
    ones_d = nc.declare_dram_parameter("ones", [128, 1], F32R, isOutput=False)
    out = nc.declare_dram_parameter("out", [C, S], F32, isOutput=True)

    Exp = mybir.ActivationFunctionType.Exp

    with PatchedTileContext(nc) as tc, ExitStack() as ctx:
        const = ctx.enter_context(tc.tile_pool(name="const", bufs=1))
        inp = ctx.enter_context(tc.tile_pool(name="inp", bufs=1))
        epool = ctx.enter_context(tc.tile_pool(name="epool", bufs=1))

        # ---- input loads ----
        bT_sb = inp.tile([D, S], F32R, tag="bT")
        nc.sync.dma_start(out=bT_sb, in_=bT[:, :])
        cT_sb = inp.tile([D, S], F32R, tag="cT")
        nc.sync.dma_start(out=cT_sb, in_=cT[:, :])
        aT_sb = inp.tile([128, MT, C], F32R, tag="aT")
        nc.sync.dma_start(out=aT_sb, in_=aT.rearrange("(t p) c -> p t c", p=128))
        a32_sb = inp.tile([128, C // 128, S], F32, tag="a32")
        nc.sync.dma_start(out=a32_sb, in_=a32.rearrange("(t p) n -> p t n", p=128))

        ones_col = const.tile([128, 1], F32R, tag="ones_col")  # K=128, M=1
        nc.sync.dma_start(out=ones_col, in_=ones_d[:, :])
        ones_row = const.tile([1, 128], F32, tag="ones_row")  # K=1, M=128
        nc.vector.memset(ones_row, 1.0)

        # persistent E^T tiles (16 x [128, S])
        e_tiles = []
        for mt in range(MT):
            e_tiles.append(epool.tile([128, S], E_DT, name=f"e{mt}", tag=f"e{mt}"))

        # ---- phase 1: scores^T -> exp ----
        with tc.tile_pool(name="scps", bufs=2, space="PSUM") as scps:
            for mt in range(MT):
                sc = scps.tile([128, S], F32, tag="sc")  # 4 banks
                for j in range(NCH):
                    nc.tensor.matmul(
                        sc[:, j * 512 : (j + 1) * 512],
                        lhsT=cT_sb[:, mt * 128 : (mt + 1) * 128],
                        rhs=bT_sb[:, j * 512 : (j + 1) * 512],
                        start=True,
                        stop=True,
                    )
                nc.scalar.activation(e_tiles[mt][:, :], sc[:, :], Exp)

        # ---- phase 2: out2 = aT.T @ E, Z = ones.T @ E, finalize ----
        with (
            tc.tile_pool(name="o2ps", bufs=2, space="PSUM") as o2ps,
            tc.tile_pool(name="zps", bufs=2, space="PSUM") as zps,
            tc.tile_pool(name="rbps", bufs=2, space="PSUM") as rbps,
            tc.tile_pool(name="fin", bufs=2) as fin,
            tc.tile_pool(name="outp", bufs=3) as outp,
        ):
            out_r = out.rearrange("(t p) n -> p t n", p=128)
            for j in range(NCH):
                js = slice(j * 512, (j + 1) * 512)
                o2_0 = o2ps.tile([128, 512], F32, tag="o2a")
                o2_1 = o2ps.tile([128, 512], F32, tag="o2b")
                z = zps.tile([1, 512], F32, tag="z")
                for mt in range(MT):
                    st = mt == 0
                    sp = mt == MT - 1
                    ejs = e_tiles[mt][:, js]
                    nc.tensor.matmul(
                        o2_0,
                        lhsT=aT_sb[:, mt, 0:128],
                        rhs=ejs,
                        start=st,
                        stop=sp,
                    )
                    nc.tensor.matmul(
                        o2_1,
                        lhsT=aT_sb[:, mt, 128:256],
                        rhs=ejs,
                        start=st,
                        stop=sp,
                    )
                    nc.tensor.matmul(
                        z, lhsT=ones_col[:, 0:1], rhs=ejs, start=st, stop=sp
                    )
                r = fin.tile([1, 512], F32, tag="r")
                nc.vector.reciprocal(r, z[0:1, :])
                rb = rbps.tile([128, 512], F32, tag="rb")
                nc.tensor.matmul(
                    rb,
                    lhsT=ones_row[0:1, :],
                    rhs=r[0:1, :],
                    start=True,
                    stop=True,
                )
                rb_sb = fin.tile([128, 512], F32, tag="rb_sb")
                nc.scalar.copy(rb_sb, rb[:, :])
                for ct in range(C // 128):
                    o2 = o2_0 if ct == 0 else o2_1
                    t1 = fin.tile([128, 512], F32, tag="t1")
                    nc.vector.tensor_mul(t1, o2[:, :], rb_sb)
                    o_sb = outp.tile([128, 512], F32, tag="o_sb")
                    nc.vector.tensor_add(o_sb, t1, a32_sb[:, ct, js])
                    nc.sync.dma_start(out=out_r[:, ct, js], in_=o_sb)

    orig_to_json_bytes = nc.to_json_bytes

    def to_json_bytes():
        return _split_sync_waits_json(orig_to_json_bytes())

    nc.to_json_bytes = to_json_bytes
    return nc


_NC_CACHE = None


def _get_nc():
    global _NC_CACHE
    if _NC_CACHE is None:
        _NC_CACHE = build_nc()
    return _NC_CACHE


def kernel(a, b, c, **run_kwargs):
    """a: [8, 256, 2048] f32, b: [8, 2048, 64] f32, c: [8, 2048, 64] f32
    -> [8, 256, 2048] f32"""
    a = np.asarray(a, dtype=np.float32)
    b = np.asarray(b, dtype=np.float32)
    c = np.asarray(c, dtype=np.float32)
    in_maps = []
    for i in range(N_CORES):
        in_maps.append(
            {
                "bT": np.ascontiguousarray(b[i].T),
                "cT": np.ascontiguousarray(c[i].T),
                "aT": np.ascontiguousarray(a[i].T).astype(E_NP),
                "a32": np.ascontiguousarray(a[i]),
                "ones": np.ones((128, 1), dtype=np.float32),
            }
        )
    res = run_bass_kernel_spmd(_get_nc(), in_maps, list(range(N_CORES)), **run_kwargs)
    out = np.stack([np.asarray(res.results[i]["out"]) for i in range(N_CORES)])
    if run_kwargs:
        kernel.last_result = res
    return out.astype(np.float32)
